# revision 31
# baseline (speedup 1.0000x reference)
"""3-layer GCN (PyG-style GCNConv with self-loops + symmetric norm) on 8
Trainium2 NeuronCores.

Distribution (1D graph partitioning):
  - nodes split into 8 contiguous blocks of 6250 rows, one per core
  - edges partitioned by destination core, sorted by destination node
  - 256x256 weights replicated on every core

The symmetric norm dinv[src]*dinv[dst] is factored out of the per-edge
selection matrix: the src factor is folded into the gathered table rows
(table[s] = dinv[s] * y[s], applied for free by the ACT-engine copy that
drains the GEMM PSUM), and the dst factor is applied by the ACT-engine
ReLU epilogue (activation scale operand).  The selection matrix is then a
pure one-hot built in a single DVE is_equal pass.

The gather table is split into two Shared tensors by source row: the A
half (each core's local rows [0, 4096) = chunks 0..31, 8*4096 = 32768
rows — the exact int16 index limit) and the B half (rows [4096, 6250),
8*2154 rows).  Each half is filled by its own single-writer AllGather:
AG_A fires once GEMM chunk 31 is done (~2/3 through the previous phase,
fully hidden), AG_B at phase end.  The next layer's A-stream gathers are
emitted several chunks ahead of its B-stream gathers, so AG_B's flight
is covered by A-gather work instead of a pipeline bubble.

Per layer, per core (software-pipelined emission, per chunk):
  stage A: dma_gather of table[src] rows for the chunk's A/B edge tiles
     (int16 indices, contiguous fp16 rows, G*128 rows per instruction,
     SWDGE queues round-robin) + one-hot eq build (DVE is_equal)
  stage B: PSUM-accumulated fp16 matmuls ps += eqT.T @ msg
  stage C: relu(dinv_dst * ps [+ bias]) on ACT, residual add, next-layer
     GEMM for the chunk (PE transposes + fp16 matmuls + scaled ACT copy),
     and the half-table AllGathers at chunks 31 / 48
"""

import math
import os

import numpy as np

import concourse.bass as bass
import concourse.mybir as mybir
import concourse.tile as tile
from concourse import bacc
from concourse.bass_utils import run_bass_kernel_spmd
from concourse.masks import make_identity

F32 = mybir.dt.float32
F16 = mybir.dt.float16
I16 = mybir.dt.int16
I32 = mybir.dt.int32

N_NODES = 50000
HID = 256
NCORES = 8
NPC = N_NODES // NCORES          # 6250 nodes per core
NCHUNK = math.ceil(NPC / 128)    # 49 dst chunks per core
G = 8                            # edge tiles per gather instruction (dma_gather tops out at 1024 idxs)
PAD_DST = 255.0                  # dst_local sentinel that matches no iota lane
NLAYERS = 3
NSWDGE_QUEUES = 4                # parallel SWDGE descriptor-gen queues
MM_DT = mybir.dt.float16         # table/message/matmul/h dtype (PSUM accum stays fp32)
CSPLIT = 32                      # chunk where the A|B table halves split
AROWS = CSPLIT * 128             # 4096 rows/core in the A half (8*4096 = int16 limit)
BROWS = NPC - AROWS              # 2154 rows/core in the B half
LAG_B = 7                        # chunks the B gather stream trails the A stream
DC = 2                           # chunks the epilogue trails the matmul stage
NBUF_A = 13                      # msg/eq ring depth, A stream
NBUF_B = 4                       # msg/eq ring depth, B stream

_cache = {}


def _pack_stream(flat_idx, flat_dst, NG):
    """flat_* are [NG*G*128] slot arrays in (tile, slot) order.

    Returns packed meta [NG*128, G*5] int32 rows: [G*8 int16 idx | G f32 dst].
    """
    dstT = (
        flat_dst.reshape(NG, G, 128).transpose(0, 2, 1).reshape(NG * 128, G)
    )
    idxT = np.zeros((NG * 128, G * 8), dtype=np.int16)
    vals = flat_idx.reshape(NG, G * 128)
    for g in range(NG):
        a16 = vals[g].reshape(G * 8, 16).T  # [16, G*8]; slot i at [i%16, i//16]
        idxT[g * 128 : (g + 1) * 128] = np.tile(a16, (8, 1))
    meta = np.zeros((NG * 128, G * 4 + G), dtype=np.int32)
    meta[:, : G * 4] = idxT.view(np.int32)
    meta[:, G * 4 : G * 5] = dstT.astype(np.float32).view(np.int32)
    return (meta,)


def _preprocess(edge_index):
    """Edge partitioning by destination + per-core A|B-half stream layouts."""
    src = np.asarray(edge_index[0], dtype=np.int64)
    dst = np.asarray(edge_index[1], dtype=np.int64)
    loops = np.arange(N_NODES, dtype=np.int64)
    s = np.concatenate([src, loops])
    d = np.concatenate([dst, loops])
    deg = np.bincount(d, minlength=N_NODES).astype(np.float32)
    dinv = (1.0 / np.sqrt(np.maximum(deg, 1e-12))).astype(np.float32)
    dinv[deg == 0] = 0.0

    r_src = s % NPC
    c_src = s // NPC
    in_a = r_src < AROWS
    idx_all = np.where(in_a, c_src * AROWS + r_src, c_src * BROWS + (r_src - AROWS))

    edges = []  # [core][chunk] -> ((idxA, dstA), (idxB, dstB))
    cntA = np.zeros((NCORES, NCHUNK), dtype=np.int64)
    cntB = np.zeros((NCORES, NCHUNK), dtype=np.int64)
    for c in range(NCORES):
        lo = c * NPC
        m = (d >= lo) & (d < lo + NPC)
        cs, cd, ca = idx_all[m], (d[m] - lo), in_a[m]
        order = np.argsort(cd, kind="stable")
        cs, cd, ca = cs[order], cd[order], ca[order]
        bounds = np.searchsorted(cd, np.arange(0, NCHUNK + 1) * 128)
        rows = []
        for ch in range(NCHUNK):
            a, b = bounds[ch], bounds[ch + 1]
            es, ed, ea = cs[a:b], cd[a:b] - ch * 128, ca[a:b]
            av = (es[ea], ed[ea])
            bv = (es[~ea], ed[~ea])
            rows.append((av, bv))
            cntA[c, ch] = ea.sum()
            cntB[c, ch] = (~ea).sum()
        edges.append(rows)

    TA = [int(np.ceil(max(cntA[:, ch].max(), 1) / 128)) for ch in range(NCHUNK)]
    TB = [int(np.ceil(max(cntB[:, ch].max(), 1) / 128)) for ch in range(NCHUNK)]
    tilesA, tilesB = int(np.sum(TA)), int(np.sum(TB))
    NGA, NGB = math.ceil(tilesA / G), math.ceil(tilesB / G)
    startA = np.concatenate([[0], np.cumsum(TA)]).astype(int)
    startB = np.concatenate([[0], np.cumsum(TB)]).astype(int)

    per_core = []
    for c in range(NCORES):
        fiA = np.zeros(NGA * G * 128, dtype=np.int64)  # pad idx: row 0
        fdA = np.full(NGA * G * 128, PAD_DST, dtype=np.float32)
        fiB = np.zeros(NGB * G * 128, dtype=np.int64)
        fdB = np.full(NGB * G * 128, PAD_DST, dtype=np.float32)
        for ch in range(NCHUNK):
            (ais, ads), (bis, bds) = edges[c][ch]
            p0 = startA[ch] * 128
            fiA[p0 : p0 + len(ais)] = ais
            fdA[p0 : p0 + len(ads)] = ads
            p0 = startB[ch] * 128
            fiB[p0 : p0 + len(bis)] = bis
            fdB[p0 : p0 + len(bds)] = bds
        per_core.append(_pack_stream(fiA, fdA, NGA) + _pack_stream(fiB, fdB, NGB))

    # per-core dinv, laid out [128, NCHUNK] column-per-chunk
    dinv_cols = np.zeros((NCORES, 128, NCHUNK), dtype=np.float32)
    for c in range(NCORES):
        dv = dinv[c * NPC : (c + 1) * NPC]
        pad = np.zeros(NCHUNK * 128, dtype=np.float32)
        pad[: len(dv)] = dv
        dinv_cols[c] = pad.reshape(NCHUNK, 128).T

    sched = (tuple(TA), tuple(TB), tilesA, tilesB, NGA, NGB)
    return sched, per_core, dinv_cols


def _build(sched, nlayers=3, has_bias=False):
    TA, TB, tilesA, tilesB, NGA, NGB = sched
    nc = bacc.Bacc(
        "TRN2",
        target_bir_lowering=False,
        debug=False,
        num_devices=NCORES,
        num_swdge_queues=NSWDGE_QUEUES,
    )
    x_ap = nc.dram_tensor("x", [NPC, HID], MM_DT, kind="ExternalInput").ap()
    wts = nc.dram_tensor(
        "wts", [2 * nlayers, 128, HID], MM_DT, kind="ExternalInput"
    ).ap()
    bias = nc.dram_tensor("bias", [nlayers, HID], F32, kind="ExternalInput").ap()
    consts = nc.dram_tensor("consts", [128, 128], F32, kind="ExternalInput").ap()
    dinv_ap = nc.dram_tensor(
        "dinv", [128, NCHUNK], F32, kind="ExternalInput"
    ).ap()
    G5 = G * 5
    metA = nc.dram_tensor(
        "metA", [NGA * 128, G5], I32, kind="ExternalInput"
    ).ap()
    metB = nc.dram_tensor(
        "metB", [NGB * 128, G5], I32, kind="ExternalInput"
    ).ap()
    out_ap = nc.dram_tensor("out", [NPC, HID], F32, kind="ExternalOutput").ap()

    with tile.TileContext(nc) as tc:
        with tc.tile_pool(name="const", bufs=1) as cpool, \
             tc.tile_pool(name="work", bufs=4) as work, \
             tc.tile_pool(name="msgA", bufs=NBUF_A) as msgA, \
             tc.tile_pool(name="msgB", bufs=NBUF_B) as msgB, \
             tc.tile_pool(name="eqA", bufs=NBUF_A) as eqA, \
             tc.tile_pool(name="eqB", bufs=NBUF_B) as eqB, \
             tc.tile_pool(name="ptp", bufs=1, space="PSUM") as ptp, \
             tc.tile_pool(name="ypp", bufs=1, space="PSUM") as ypp, \
             tc.tile_pool(name="psp", bufs=6, space="PSUM") as psp, \
             tc.tile_pool(name="dram", bufs=1, space="DRAM") as dram:

            identity = cpool.tile([128, 128], MM_DT)
            make_identity(nc, identity[:])
            iota_sb = cpool.tile([128, 128], F32)
            nc.sync.dma_start(out=iota_sb[:], in_=consts[:])
            dinv_sb = cpool.tile([128, NCHUNK], F32)
            nc.sync.dma_start(out=dinv_sb[:], in_=dinv_ap[:])

            # whole meta resident in SBUF (single big DMA per stream):
            # gathers/eq builds then have zero runtime DMA dependencies
            met_all = {
                "A": cpool.tile([128, NGA * G5], I32, name="metA_sb"),
                "B": cpool.tile([128, NGB * G5], I32, name="metB_sb"),
            }
            for sname, met_d, ng in (("A", metA, NGA), ("B", metB, NGB)):
                t = met_all[sname]
                nc.sync.dma_start(
                    out=t[:].rearrange("p (g c) -> p g c", g=ng),
                    in_=met_d[:].rearrange("(g p) c -> p g c", g=ng),
                )

            wt_sb = cpool.tile([128, 2 * nlayers * HID], MM_DT)
            for i in range(2 * nlayers):
                nc.sync.dma_start(
                    out=wt_sb[:, i * HID : (i + 1) * HID], in_=wts[i]
                )

            if has_bias:
                bias_row = cpool.tile([1, nlayers * HID], F32)
                ones_sb = cpool.tile([1, 128], F32)
                nc.vector.memset(ones_sb[:], 1.0)
                bias128 = cpool.tile([128, nlayers * HID], F32)
                for l in range(nlayers):
                    nc.sync.dma_start(
                        out=bias_row[:, l * HID : (l + 1) * HID],
                        in_=bias[l : l + 1, :],
                    )
                    bp = ptp.tile([128, HID], F32, tag="pt", name="pt")
                    nc.tensor.matmul(
                        out=bp[:, :],
                        lhsT=ones_sb[:],
                        rhs=bias_row[:, l * HID : (l + 1) * HID],
                        start=True,
                        stop=True,
                    )
                    nc.scalar.copy(
                        out=bias128[:, l * HID : (l + 1) * HID], in_=bp[:, :]
                    )

            # zero-init the msg rings so pad slots never feed NaN bit
            # patterns into the 0-weighted matmul columns
            for pool, n in ((msgA, NBUF_A), (msgB, NBUF_B)):
                for _ in range(n):
                    mz = pool.tile([128, G * HID], MM_DT, tag="msg", name="msg")
                    nc.vector.memset(mz[:], 0)

            # h lives in SBUF as one big tile (single DMA load), fp16
            h_all = cpool.tile([128, NCHUNK * HID], MM_DT, name="h_all")
            nc.sync.dma_start(
                out=h_all[:].rearrange("p (c d) -> p c d", c=NCHUNK)[
                    :, : NPC // 128, :
                ],
                in_=x_ap[: (NPC // 128) * 128, :].rearrange(
                    "(c p) d -> p c d", c=NPC // 128
                ),
            )
            tail = NPC - (NPC // 128) * 128
            if tail:
                nc.sync.dma_start(
                    out=h_all[:tail, (NCHUNK - 1) * HID : NCHUNK * HID],
                    in_=x_ap[(NPC // 128) * 128 :, :],
                )
            h_sb = [h_all[:, c * HID : (c + 1) * HID] for c in range(NCHUNK)]

            y_cs = [
                {
                    "A": dram.tile([AROWS, HID], MM_DT, name=f"y_cA{i}"),
                    "B": dram.tile([BROWS, HID], MM_DT, name=f"y_cB{i}"),
                }
                for i in range(nlayers)
            ]
            y_tabs = [
                {
                    "A": dram.tile(
                        [AROWS * NCORES, HID],
                        MM_DT,
                        addr_space="Shared",
                        name=f"y_tabA{i}",
                    ),
                    "B": dram.tile(
                        [BROWS * NCORES, HID],
                        MM_DT,
                        addr_space="Shared",
                        name=f"y_tabB{i}",
                    ),
                }
                for i in range(nlayers)
            ]

            def gemm_chunk(l, c):
                """layer-l GEMM chunk c: y rows = dinv * (h_sb[c] @ W_l.T)"""
                rows = min(128, NPC - c * 128)
                hT = work.tile([128, HID], MM_DT, tag="hT", name="hT")
                for k in range(2):
                    pt = ptp.tile([128, 128], MM_DT, tag="pt", name="pt")
                    nc.tensor.transpose(
                        out=pt[:, :rows],
                        in_=h_sb[c][:rows, k * 128 : (k + 1) * 128],
                        identity=identity[:rows, :rows],
                    )
                    nc.scalar.copy(
                        out=hT[:, k * 128 : k * 128 + rows], in_=pt[:, :rows]
                    )
                yp = ypp.tile([128, HID], F32, tag="yp", name="yp")
                for k in range(2):
                    nc.tensor.matmul(
                        out=yp[:rows, :],
                        lhsT=hT[:, k * 128 : k * 128 + rows],
                        rhs=wt_sb[:, (2 * l + k) * HID : (2 * l + k + 1) * HID],
                        start=(k == 0),
                        stop=(k == 1),
                    )
                y_sb = work.tile([128, HID], MM_DT, tag="y_sb", name="y_sb")
                nc.scalar.activation(
                    out=y_sb[:rows],
                    in_=yp[:rows, :],
                    func=mybir.ActivationFunctionType.Copy,
                    scale=dinv_sb[:rows, c : c + 1],
                )
                if c < CSPLIT:
                    nc.sync.dma_start(
                        out=y_cs[l]["A"][c * 128 : c * 128 + rows, :],
                        in_=y_sb[:rows],
                    )
                else:
                    base = (c - CSPLIT) * 128
                    nc.sync.dma_start(
                        out=y_cs[l]["B"][base : base + rows, :], in_=y_sb[:rows]
                    )

            def allgather(l, half):
                nc.gpsimd.collective_compute(
                    "AllGather",
                    mybir.AluOpType.bypass,
                    replica_groups=[list(range(NCORES))],
                    ins=[y_cs[l][half][:].opt()],
                    outs=[y_tabs[l][half][:].opt()],
                )

            for ci in range(NCHUNK):
                gemm_chunk(0, ci)
                if ci == CSPLIT - 1:
                    allgather(0, "A")
            allgather(0, "B")

            startA_l = [0]
            for t in TA:
                startA_l.append(startA_l[-1] + t)
            startB_l = [0]
            for t in TB:
                startB_l.append(startB_l[-1] + t)

            for l in range(nlayers):
                stream_info = {
                    "A": (tilesA, NGA, startA_l, y_tabs[l]["A"][:], msgA, eqA),
                    "B": (tilesB, NGB, startB_l, y_tabs[l]["B"][:], msgB, eqB),
                }
                nextg = {"A": 0, "B": 0}
                gbufs = {"A": {}, "B": {}}
                ps_of = {}
                qctr = [0]

                def stage_a(ci, sname):
                    tiles_s, ng_s, starts, view, mpool, epool = stream_info[sname]
                    while (
                        nextg[sname] < ng_s
                        and nextg[sname] * G < starts[ci + 1]
                    ):
                        g = nextg[sname]
                        rem = min(G, tiles_s - g * G)
                        met_sb = met_all[sname][:, g * G5 : (g + 1) * G5]
                        idx_sb = met_sb[:, : G * 4].bitcast(I16)
                        dst_sb = met_sb[:, G * 4 : G * 5].bitcast(F32)
                        msg = mpool.tile(
                            [128, G * HID], MM_DT, tag="msg", name="msg"
                        )
                        nc.gpsimd.dma_gather(
                            out_ap=msg[:, : rem * HID].rearrange(
                                "p (g d) -> p g d", g=rem
                            ),
                            in_ap=view,
                            idxs_ap=idx_sb[:, : rem * 8],
                            num_idxs=rem * 128,
                            num_idxs_reg=rem * 128,
                            elem_size=HID,
                            queue_num=qctr[0] % NSWDGE_QUEUES,
                        )
                        qctr[0] += 1
                        eq = epool.tile(
                            [128, G * 128], MM_DT, tag="eq", name="eq"
                        )
                        eq3 = eq[:, : rem * 128].rearrange(
                            "p (g d) -> p g d", g=rem
                        )
                        nc.vector.tensor_tensor(
                            out=eq3,
                            in0=dst_sb[:, :rem, None].to_broadcast(
                                (128, rem, 128)
                            ),
                            in1=iota_sb[:, None, :].to_broadcast(
                                (128, rem, 128)
                            ),
                            op=mybir.AluOpType.is_equal,
                        )
                        gbufs[sname][g] = (msg, eq)
                        nextg[sname] += 1

                def stage_b(ci):
                    ntot = TA[ci] + TB[ci]
                    ps = psp.tile([128, HID], F32, tag="ps", name="ps")
                    jj = 0
                    for sname, T_s, starts in (
                        ("A", TA, startA_l),
                        ("B", TB, startB_l),
                    ):
                        for t in range(T_s[ci]):
                            g, col = divmod(starts[ci] + t, G)
                            msg, eq = gbufs[sname][g]
                            nc.tensor.matmul(
                                out=ps[:, :],
                                lhsT=eq[:, col * 128 : (col + 1) * 128],
                                rhs=msg[:, col * HID : (col + 1) * HID],
                                start=(jj == 0),
                                stop=(jj == ntot - 1),
                            )
                            jj += 1
                    ps_of[ci] = ps

                def stage_c(ci):
                    crows = min(128, NPC - ci * 128)
                    ps = ps_of.pop(ci)
                    if has_bias:
                        t_sb = work.tile([128, HID], F32, tag="o_sb", name="t_sb")
                        nc.vector.scalar_tensor_tensor(
                            out=t_sb[:crows],
                            in0=ps[:crows, :],
                            scalar=dinv_sb[:crows, ci : ci + 1],
                            in1=bias128[:crows, l * HID : (l + 1) * HID],
                            op0=mybir.AluOpType.mult,
                            op1=mybir.AluOpType.add,
                        )
                        relu_in, relu_scale = t_sb, 1.0
                    else:
                        relu_in, relu_scale = ps, dinv_sb[:crows, ci : ci + 1]
                    if l == 0:
                        nc.scalar.activation(
                            out=h_sb[ci][:crows],
                            in_=relu_in[:crows, :],
                            func=mybir.ActivationFunctionType.Relu,
                            scale=relu_scale,
                        )
                    else:
                        odt = F32 if l == nlayers - 1 else MM_DT
                        o_sb = work.tile([128, HID], odt, tag="o_sb", name="o_sb")
                        nc.scalar.activation(
                            out=o_sb[:crows],
                            in_=relu_in[:crows, :],
                            func=mybir.ActivationFunctionType.Relu,
                            scale=relu_scale,
                        )
                        if l < nlayers - 1:
                            nc.vector.tensor_add(
                                out=h_sb[ci][:crows],
                                in0=o_sb[:crows],
                                in1=h_sb[ci][:crows],
                            )
                        else:
                            nc.vector.tensor_add(
                                out=o_sb[:crows],
                                in0=o_sb[:crows],
                                in1=h_sb[ci][:crows],
                            )
                            nc.sync.dma_start(
                                out=out_ap[ci * 128 : ci * 128 + crows, :],
                                in_=o_sb[:crows],
                            )
                    if l + 1 < nlayers:
                        gemm_chunk(l + 1, ci)
                        if ci == CSPLIT - 1:
                            allgather(l + 1, "A")
                        elif ci == NCHUNK - 1:
                            allgather(l + 1, "B")

                NIT = NCHUNK + LAG_B + 1 + DC
                for it in range(NIT):
                    if it < NCHUNK:
                        stage_a(it, "A")
                    if LAG_B <= it < NCHUNK + LAG_B:
                        stage_a(it - LAG_B, "B")
                    if LAG_B + 1 <= it < NCHUNK + LAG_B + 1:
                        stage_b(it - LAG_B - 1)
                    if it >= LAG_B + 1 + DC:
                        stage_c(it - LAG_B - 1 - DC)

    nc.compile()
    return nc


def _consts_array():
    consts = np.zeros((128, 128), dtype=np.float32)
    consts[:, :] = np.arange(128, dtype=np.float32)[None, :]
    return consts


def kernel(x, edge_index, W0, b0, W1, b1, W2, b2):
    x = np.asarray(x, dtype=np.float32)
    edge_index = np.asarray(edge_index)
    Ws = [np.asarray(w, dtype=np.float32) for w in (W0, W1, W2)]
    bs = [np.asarray(b, dtype=np.float32) for b in (b0, b1, b2)]
    has_bias = any(np.any(b != 0) for b in bs)

    sched, per_core, dinv_cols = _preprocess(edge_index)

    key = (sched, NLAYERS, has_bias)
    if key not in _cache:
        _cache[key] = _build(sched, nlayers=NLAYERS, has_bias=has_bias)
    nc = _cache[key]

    wts = np.stack(
        [w.T[k * 128 : (k + 1) * 128, :] for w in Ws for k in range(2)]
    ).astype(np.float16)
    bias_arr = np.stack(bs).astype(np.float32)
    consts = _consts_array()

    in_maps = []
    for c in range(NCORES):
        mA, mB = per_core[c]
        in_maps.append(
            {
                "x": np.ascontiguousarray(x[c * NPC : (c + 1) * NPC]).astype(np.float16),
                "wts": wts,
                "bias": bias_arr,
                "consts": consts,
                "dinv": np.ascontiguousarray(dinv_cols[c]),
                "metA": mA,
                "metB": mB,
            }
        )

    trace = bool(int(os.environ.get("GCN_TRACE", "0")))
    res = run_bass_kernel_spmd(
        nc, in_maps, core_ids=list(range(NCORES)), trace=trace
    )
    if trace:
        kernel.last_exec_time_ns = res.exec_time_ns
        kernel.last_results = res
    out = np.concatenate([res.results[c]["out"] for c in range(NCORES)], axis=0)
    return out


# revision 32
# speedup vs baseline: 1.2312x; 1.2312x over previous
"""3-layer GCN (PyG-style GCNConv with self-loops + symmetric norm) on 8
Trainium2 NeuronCores.

Distribution (1D graph partitioning):
  - nodes split into 8 contiguous blocks of 6250 rows, one per core
  - edges partitioned by destination core, sorted by destination node
  - 256x256 weights replicated on every core

The symmetric norm dinv[src]*dinv[dst] is factored out of the per-edge
selection matrix: the src factor is folded into the gathered table rows
(table[s] = dinv[s] * y[s], applied for free by the ACT-engine copy that
drains the GEMM PSUM), and the dst factor is applied by the ACT-engine
ReLU epilogue (activation scale operand).  The selection matrix is then a
pure one-hot built in a single DVE is_equal pass.

The gather table is split into two Shared tensors by source row: the A
half (each core's local rows [0, 4096) = chunks 0..31, 8*4096 = 32768
rows — the exact int16 index limit) and the B half (rows [4096, 6250),
8*2154 rows).  Each half is filled by its own single-writer AllGather:
AG_A fires once GEMM chunk 31 is done (~2/3 through the previous phase,
fully hidden), AG_B at phase end.  The next layer's A-stream gathers are
emitted several chunks ahead of its B-stream gathers, so AG_B's flight
is covered by A-gather work instead of a pipeline bubble.

Per layer, per core (software-pipelined emission, per chunk):
  stage A: dma_gather of table[src] rows for the chunk's A/B edge tiles
     (int16 indices, contiguous fp16 rows, G*128 rows per instruction,
     SWDGE queues round-robin) + one-hot eq build (DVE is_equal)
  stage B: PSUM-accumulated fp16 matmuls ps += eqT.T @ msg
  stage C: relu(dinv_dst * ps [+ bias]) on ACT, residual add, next-layer
     GEMM for the chunk (PE transposes + fp16 matmuls + scaled ACT copy),
     and the half-table AllGathers at chunks 31 / 48
"""

import math
import os

import numpy as np

import concourse.bass as bass
import concourse.mybir as mybir
import concourse.tile as tile
from concourse import bacc
from concourse.bass_utils import run_bass_kernel_spmd
from concourse.masks import make_identity

F32 = mybir.dt.float32
F16 = mybir.dt.float16
I16 = mybir.dt.int16
I32 = mybir.dt.int32

N_NODES = 50000
HID = 256
NCORES = 8
NPC = N_NODES // NCORES          # 6250 nodes per core
NCHUNK = math.ceil(NPC / 128)    # 49 dst chunks per core
G = 8                            # edge tiles per gather instruction (dma_gather tops out at 1024 idxs)
PAD_DST = 255.0                  # dst_local sentinel that matches no iota lane
NLAYERS = 3
NSWDGE_QUEUES = 4                # parallel SWDGE descriptor-gen queues
MM_DT = mybir.dt.float16         # table/message/matmul/h dtype (PSUM accum stays fp32)
DC = 2                           # chunks the epilogue trails the matmul stage
NBUF_A = 10                      # msg/eq ring depth, even stream
NBUF_B = 10                      # msg/eq ring depth, odd stream

_cache = {}


def _pack_stream(flat_idx, flat_dst, NG):
    """flat_* are [NG*G*128] slot arrays in (tile, slot) order.

    Returns packed meta [NG*128, G*5] int32 rows: [G*8 int16 idx | G f32 dst].
    """
    dstT = (
        flat_dst.reshape(NG, G, 128).transpose(0, 2, 1).reshape(NG * 128, G)
    )
    idxT = np.zeros((NG * 128, G * 8), dtype=np.int16)
    vals = flat_idx.reshape(NG, G * 128)
    for g in range(NG):
        a16 = vals[g].reshape(G * 8, 16).T  # [16, G*8]; slot i at [i%16, i//16]
        idxT[g * 128 : (g + 1) * 128] = np.tile(a16, (8, 1))
    meta = np.zeros((NG * 128, G * 4 + G), dtype=np.int32)
    meta[:, : G * 4] = idxT.view(np.int32)
    meta[:, G * 4 : G * 5] = dstT.astype(np.float32).view(np.int32)
    return (meta,)


def _preprocess(edge_index):
    """Edge partitioning by destination + per-core A|B-half stream layouts."""
    src = np.asarray(edge_index[0], dtype=np.int64)
    dst = np.asarray(edge_index[1], dtype=np.int64)
    loops = np.arange(N_NODES, dtype=np.int64)
    s = np.concatenate([src, loops])
    d = np.concatenate([dst, loops])
    deg = np.bincount(d, minlength=N_NODES).astype(np.float32)
    dinv = (1.0 / np.sqrt(np.maximum(deg, 1e-12))).astype(np.float32)
    dinv[deg == 0] = 0.0

    in_a = (s % 2) == 0
    idx_all = s // 2

    edges = []  # [core][chunk] -> ((idxA, dstA), (idxB, dstB))
    cntA = np.zeros((NCORES, NCHUNK), dtype=np.int64)
    cntB = np.zeros((NCORES, NCHUNK), dtype=np.int64)
    for c in range(NCORES):
        lo = c * NPC
        m = (d >= lo) & (d < lo + NPC)
        cs, cd, ca = idx_all[m], (d[m] - lo), in_a[m]
        order = np.argsort(cd, kind="stable")
        cs, cd, ca = cs[order], cd[order], ca[order]
        bounds = np.searchsorted(cd, np.arange(0, NCHUNK + 1) * 128)
        rows = []
        for ch in range(NCHUNK):
            a, b = bounds[ch], bounds[ch + 1]
            es, ed, ea = cs[a:b], cd[a:b] - ch * 128, ca[a:b]
            av = (es[ea], ed[ea])
            bv = (es[~ea], ed[~ea])
            rows.append((av, bv))
            cntA[c, ch] = ea.sum()
            cntB[c, ch] = (~ea).sum()
        edges.append(rows)

    TA = [int(np.ceil(max(cntA[:, ch].max(), 1) / 128)) for ch in range(NCHUNK)]
    TB = [int(np.ceil(max(cntB[:, ch].max(), 1) / 128)) for ch in range(NCHUNK)]
    tilesA, tilesB = int(np.sum(TA)), int(np.sum(TB))
    NGA, NGB = math.ceil(tilesA / G), math.ceil(tilesB / G)
    startA = np.concatenate([[0], np.cumsum(TA)]).astype(int)
    startB = np.concatenate([[0], np.cumsum(TB)]).astype(int)

    per_core = []
    for c in range(NCORES):
        fiA = np.zeros(NGA * G * 128, dtype=np.int64)  # pad idx: row 0
        fdA = np.full(NGA * G * 128, PAD_DST, dtype=np.float32)
        fiB = np.zeros(NGB * G * 128, dtype=np.int64)
        fdB = np.full(NGB * G * 128, PAD_DST, dtype=np.float32)
        for ch in range(NCHUNK):
            (ais, ads), (bis, bds) = edges[c][ch]
            p0 = startA[ch] * 128
            fiA[p0 : p0 + len(ais)] = ais
            fdA[p0 : p0 + len(ads)] = ads
            p0 = startB[ch] * 128
            fiB[p0 : p0 + len(bis)] = bis
            fdB[p0 : p0 + len(bds)] = bds
        per_core.append(_pack_stream(fiA, fdA, NGA) + _pack_stream(fiB, fdB, NGB))

    # per-core dinv, laid out [128, NCHUNK] column-per-chunk
    dinv_cols = np.zeros((NCORES, 128, NCHUNK), dtype=np.float32)
    for c in range(NCORES):
        dv = dinv[c * NPC : (c + 1) * NPC]
        pad = np.zeros(NCHUNK * 128, dtype=np.float32)
        pad[: len(dv)] = dv
        dinv_cols[c] = pad.reshape(NCHUNK, 128).T

    sched = (tuple(TA), tuple(TB), tilesA, tilesB, NGA, NGB)
    return sched, per_core, dinv_cols


def _build(sched, nlayers=3, has_bias=False):
    TA, TB, tilesA, tilesB, NGA, NGB = sched
    nc = bacc.Bacc(
        "TRN2",
        target_bir_lowering=False,
        debug=False,
        num_devices=NCORES,
        num_swdge_queues=NSWDGE_QUEUES,
    )
    x_ap = nc.dram_tensor("x", [NPC, HID], MM_DT, kind="ExternalInput").ap()
    wts = nc.dram_tensor(
        "wts", [2 * nlayers, 128, HID], MM_DT, kind="ExternalInput"
    ).ap()
    bias = nc.dram_tensor("bias", [nlayers, HID], F32, kind="ExternalInput").ap()
    consts = nc.dram_tensor("consts", [128, 128], F32, kind="ExternalInput").ap()
    dinv_ap = nc.dram_tensor(
        "dinv", [128, NCHUNK], F32, kind="ExternalInput"
    ).ap()
    G5 = G * 5
    metA = nc.dram_tensor(
        "metA", [NGA * 128, G5], I32, kind="ExternalInput"
    ).ap()
    metB = nc.dram_tensor(
        "metB", [NGB * 128, G5], I32, kind="ExternalInput"
    ).ap()
    out_ap = nc.dram_tensor("out", [NPC, HID], F32, kind="ExternalOutput").ap()

    with tile.TileContext(nc) as tc:
        with tc.tile_pool(name="const", bufs=1) as cpool, \
             tc.tile_pool(name="work", bufs=4) as work, \
             tc.tile_pool(name="msgA", bufs=NBUF_A) as msgA, \
             tc.tile_pool(name="msgB", bufs=NBUF_B) as msgB, \
             tc.tile_pool(name="eqA", bufs=NBUF_A) as eqA, \
             tc.tile_pool(name="eqB", bufs=NBUF_B) as eqB, \
             tc.tile_pool(name="ptp", bufs=1, space="PSUM") as ptp, \
             tc.tile_pool(name="ypp", bufs=1, space="PSUM") as ypp, \
             tc.tile_pool(name="psp", bufs=6, space="PSUM") as psp, \
             tc.tile_pool(name="dram", bufs=1, space="DRAM") as dram:

            identity = cpool.tile([128, 128], MM_DT)
            make_identity(nc, identity[:])
            iota_sb = cpool.tile([128, 128], F32)
            nc.sync.dma_start(out=iota_sb[:], in_=consts[:])
            dinv_sb = cpool.tile([128, NCHUNK], F32)
            nc.sync.dma_start(out=dinv_sb[:], in_=dinv_ap[:])

            # whole meta resident in SBUF (single big DMA per stream):
            # gathers/eq builds then have zero runtime DMA dependencies
            met_all = {
                "A": cpool.tile([128, NGA * G5], I32, name="metA_sb"),
                "B": cpool.tile([128, NGB * G5], I32, name="metB_sb"),
            }
            for sname, met_d, ng in (("A", metA, NGA), ("B", metB, NGB)):
                t = met_all[sname]
                nc.sync.dma_start(
                    out=t[:].rearrange("p (g c) -> p g c", g=ng),
                    in_=met_d[:].rearrange("(g p) c -> p g c", g=ng),
                )

            wt_sb = cpool.tile([128, 2 * nlayers * HID], MM_DT)
            for i in range(2 * nlayers):
                nc.sync.dma_start(
                    out=wt_sb[:, i * HID : (i + 1) * HID], in_=wts[i]
                )

            if has_bias:
                bias_row = cpool.tile([1, nlayers * HID], F32)
                ones_sb = cpool.tile([1, 128], F32)
                nc.vector.memset(ones_sb[:], 1.0)
                bias128 = cpool.tile([128, nlayers * HID], F32)
                for l in range(nlayers):
                    nc.sync.dma_start(
                        out=bias_row[:, l * HID : (l + 1) * HID],
                        in_=bias[l : l + 1, :],
                    )
                    bp = ptp.tile([128, HID], F32, tag="pt", name="pt")
                    nc.tensor.matmul(
                        out=bp[:, :],
                        lhsT=ones_sb[:],
                        rhs=bias_row[:, l * HID : (l + 1) * HID],
                        start=True,
                        stop=True,
                    )
                    nc.scalar.copy(
                        out=bias128[:, l * HID : (l + 1) * HID], in_=bp[:, :]
                    )

            # zero-init the msg rings so pad slots never feed NaN bit
            # patterns into the 0-weighted matmul columns
            for pool, n in ((msgA, NBUF_A), (msgB, NBUF_B)):
                for _ in range(n):
                    mz = pool.tile([128, G * HID], MM_DT, tag="msg", name="msg")
                    nc.vector.memset(mz[:], 0)

            # h lives in SBUF as one big tile (single DMA load), fp16
            h_all = cpool.tile([128, NCHUNK * HID], MM_DT, name="h_all")
            nc.sync.dma_start(
                out=h_all[:].rearrange("p (c d) -> p c d", c=NCHUNK)[
                    :, : NPC // 128, :
                ],
                in_=x_ap[: (NPC // 128) * 128, :].rearrange(
                    "(c p) d -> p c d", c=NPC // 128
                ),
            )
            tail = NPC - (NPC // 128) * 128
            if tail:
                nc.sync.dma_start(
                    out=h_all[:tail, (NCHUNK - 1) * HID : NCHUNK * HID],
                    in_=x_ap[(NPC // 128) * 128 :, :],
                )
            h_sb = [h_all[:, c * HID : (c + 1) * HID] for c in range(NCHUNK)]

            y_cs = [
                dram.tile([NPC, HID], MM_DT, name=f"y_c{i}")
                for i in range(nlayers)
            ]
            y_tabs = [
                dram.tile(
                    [NPC * NCORES, HID],
                    MM_DT,
                    addr_space="Shared",
                    name=f"y_table{i}",
                )
                for i in range(nlayers)
            ]

            def gemm_chunk(l, c):
                """layer-l GEMM chunk c: y rows = dinv * (h_sb[c] @ W_l.T)"""
                rows = min(128, NPC - c * 128)
                hT = work.tile([128, HID], MM_DT, tag="hT", name="hT")
                for k in range(2):
                    pt = ptp.tile([128, 128], MM_DT, tag="pt", name="pt")
                    nc.tensor.transpose(
                        out=pt[:, :rows],
                        in_=h_sb[c][:rows, k * 128 : (k + 1) * 128],
                        identity=identity[:rows, :rows],
                    )
                    nc.scalar.copy(
                        out=hT[:, k * 128 : k * 128 + rows], in_=pt[:, :rows]
                    )
                yp = ypp.tile([128, HID], F32, tag="yp", name="yp")
                for k in range(2):
                    nc.tensor.matmul(
                        out=yp[:rows, :],
                        lhsT=hT[:, k * 128 : k * 128 + rows],
                        rhs=wt_sb[:, (2 * l + k) * HID : (2 * l + k + 1) * HID],
                        start=(k == 0),
                        stop=(k == 1),
                    )
                y_sb = work.tile([128, HID], MM_DT, tag="y_sb", name="y_sb")
                nc.scalar.activation(
                    out=y_sb[:rows],
                    in_=yp[:rows, :],
                    func=mybir.ActivationFunctionType.Copy,
                    scale=dinv_sb[:rows, c : c + 1],
                )
                nc.sync.dma_start(
                    out=y_cs[l][c * 128 : c * 128 + rows, :], in_=y_sb[:rows]
                )

            def allgather(l):
                nc.gpsimd.collective_compute(
                    "AllGather",
                    mybir.AluOpType.bypass,
                    replica_groups=[list(range(NCORES))],
                    ins=[y_cs[l][:].opt()],
                    outs=[y_tabs[l][:].opt()],
                )

            for ci in range(NCHUNK):
                gemm_chunk(0, ci)
            allgather(0)

            startA_l = [0]
            for t in TA:
                startA_l.append(startA_l[-1] + t)
            startB_l = [0]
            for t in TB:
                startB_l.append(startB_l[-1] + t)

            for l in range(nlayers):
                stream_info = {
                    "A": (tilesA, NGA, startA_l, y_tabs[l][0::2, :], msgA, eqA),
                    "B": (tilesB, NGB, startB_l, y_tabs[l][1::2, :], msgB, eqB),
                }
                nextg = {"A": 0, "B": 0}
                gbufs = {"A": {}, "B": {}}
                ps_of = {}
                qctr = [0]

                def stage_a(ci, sname):
                    tiles_s, ng_s, starts, view, mpool, epool = stream_info[sname]
                    while (
                        nextg[sname] < ng_s
                        and nextg[sname] * G < starts[ci + 1]
                    ):
                        g = nextg[sname]
                        rem = min(G, tiles_s - g * G)
                        met_sb = met_all[sname][:, g * G5 : (g + 1) * G5]
                        idx_sb = met_sb[:, : G * 4].bitcast(I16)
                        dst_sb = met_sb[:, G * 4 : G * 5].bitcast(F32)
                        msg = mpool.tile(
                            [128, G * HID], MM_DT, tag="msg", name="msg"
                        )
                        nc.gpsimd.dma_gather(
                            out_ap=msg[:, : rem * HID].rearrange(
                                "p (g d) -> p g d", g=rem
                            ),
                            in_ap=view,
                            idxs_ap=idx_sb[:, : rem * 8],
                            num_idxs=rem * 128,
                            num_idxs_reg=rem * 128,
                            elem_size=HID,
                            elem_step=2 * HID,
                            queue_num=qctr[0] % NSWDGE_QUEUES,
                        )
                        qctr[0] += 1
                        eq = epool.tile(
                            [128, G * 128], MM_DT, tag="eq", name="eq"
                        )
                        eq3 = eq[:, : rem * 128].rearrange(
                            "p (g d) -> p g d", g=rem
                        )
                        nc.vector.tensor_tensor(
                            out=eq3,
                            in0=dst_sb[:, :rem, None].to_broadcast(
                                (128, rem, 128)
                            ),
                            in1=iota_sb[:, None, :].to_broadcast(
                                (128, rem, 128)
                            ),
                            op=mybir.AluOpType.is_equal,
                        )
                        gbufs[sname][g] = (msg, eq)
                        nextg[sname] += 1

                def stage_b(ci):
                    ntot = TA[ci] + TB[ci]
                    ps = psp.tile([128, HID], F32, tag="ps", name="ps")
                    jj = 0
                    for sname, T_s, starts in (
                        ("A", TA, startA_l),
                        ("B", TB, startB_l),
                    ):
                        for t in range(T_s[ci]):
                            g, col = divmod(starts[ci] + t, G)
                            msg, eq = gbufs[sname][g]
                            nc.tensor.matmul(
                                out=ps[:, :],
                                lhsT=eq[:, col * 128 : (col + 1) * 128],
                                rhs=msg[:, col * HID : (col + 1) * HID],
                                start=(jj == 0),
                                stop=(jj == ntot - 1),
                            )
                            jj += 1
                    ps_of[ci] = ps

                def stage_c(ci):
                    crows = min(128, NPC - ci * 128)
                    ps = ps_of.pop(ci)
                    if has_bias:
                        t_sb = work.tile([128, HID], F32, tag="o_sb", name="t_sb")
                        nc.vector.scalar_tensor_tensor(
                            out=t_sb[:crows],
                            in0=ps[:crows, :],
                            scalar=dinv_sb[:crows, ci : ci + 1],
                            in1=bias128[:crows, l * HID : (l + 1) * HID],
                            op0=mybir.AluOpType.mult,
                            op1=mybir.AluOpType.add,
                        )
                        relu_in, relu_scale = t_sb, 1.0
                    else:
                        relu_in, relu_scale = ps, dinv_sb[:crows, ci : ci + 1]
                    if l == 0:
                        nc.scalar.activation(
                            out=h_sb[ci][:crows],
                            in_=relu_in[:crows, :],
                            func=mybir.ActivationFunctionType.Relu,
                            scale=relu_scale,
                        )
                    else:
                        odt = F32 if l == nlayers - 1 else MM_DT
                        o_sb = work.tile([128, HID], odt, tag="o_sb", name="o_sb")
                        nc.scalar.activation(
                            out=o_sb[:crows],
                            in_=relu_in[:crows, :],
                            func=mybir.ActivationFunctionType.Relu,
                            scale=relu_scale,
                        )
                        if l < nlayers - 1:
                            nc.vector.tensor_add(
                                out=h_sb[ci][:crows],
                                in0=o_sb[:crows],
                                in1=h_sb[ci][:crows],
                            )
                        else:
                            nc.vector.tensor_add(
                                out=o_sb[:crows],
                                in0=o_sb[:crows],
                                in1=h_sb[ci][:crows],
                            )
                            nc.sync.dma_start(
                                out=out_ap[ci * 128 : ci * 128 + crows, :],
                                in_=o_sb[:crows],
                            )
                    if l + 1 < nlayers:
                        gemm_chunk(l + 1, ci)
                        if ci == NCHUNK - 1:
                            allgather(l + 1)

                NIT = NCHUNK + 1 + DC
                for it in range(NIT):
                    if it < NCHUNK:
                        stage_a(it, "A")
                        stage_a(it, "B")
                    if 1 <= it < NCHUNK + 1:
                        stage_b(it - 1)
                    if it >= 1 + DC:
                        stage_c(it - 1 - DC)

    nc.compile()
    return nc


def _consts_array():
    consts = np.zeros((128, 128), dtype=np.float32)
    consts[:, :] = np.arange(128, dtype=np.float32)[None, :]
    return consts


def kernel(x, edge_index, W0, b0, W1, b1, W2, b2):
    x = np.asarray(x, dtype=np.float32)
    edge_index = np.asarray(edge_index)
    Ws = [np.asarray(w, dtype=np.float32) for w in (W0, W1, W2)]
    bs = [np.asarray(b, dtype=np.float32) for b in (b0, b1, b2)]
    has_bias = any(np.any(b != 0) for b in bs)

    sched, per_core, dinv_cols = _preprocess(edge_index)

    key = (sched, NLAYERS, has_bias)
    if key not in _cache:
        _cache[key] = _build(sched, nlayers=NLAYERS, has_bias=has_bias)
    nc = _cache[key]

    wts = np.stack(
        [w.T[k * 128 : (k + 1) * 128, :] for w in Ws for k in range(2)]
    ).astype(np.float16)
    bias_arr = np.stack(bs).astype(np.float32)
    consts = _consts_array()

    in_maps = []
    for c in range(NCORES):
        mA, mB = per_core[c]
        in_maps.append(
            {
                "x": np.ascontiguousarray(x[c * NPC : (c + 1) * NPC]).astype(np.float16),
                "wts": wts,
                "bias": bias_arr,
                "consts": consts,
                "dinv": np.ascontiguousarray(dinv_cols[c]),
                "metA": mA,
                "metB": mB,
            }
        )

    trace = bool(int(os.environ.get("GCN_TRACE", "0")))
    res = run_bass_kernel_spmd(
        nc, in_maps, core_ids=list(range(NCORES)), trace=trace
    )
    if trace:
        kernel.last_exec_time_ns = res.exec_time_ns
        kernel.last_results = res
    out = np.concatenate([res.results[c]["out"] for c in range(NCORES)], axis=0)
    return out


# revision 35
# speedup vs baseline: 1.2969x; 1.0534x over previous
"""3-layer GCN (PyG-style GCNConv with self-loops + symmetric norm) on 8
Trainium2 NeuronCores.

Distribution (1D graph partitioning):
  - nodes split into 8 contiguous blocks of 6250 rows, one per core
  - edges partitioned by destination core, sorted by destination node
  - 256x256 weights replicated on every core

The symmetric norm dinv[src]*dinv[dst] is factored out of the per-edge
selection matrix: the src factor is folded into the gathered table rows
(table[s] = dinv[s] * y[s], applied for free by the ACT-engine copy that
drains the GEMM PSUM), and the dst factor is applied by the ACT-engine
ReLU epilogue (activation scale operand).  The selection matrix is then a
pure one-hot built in a single DVE is_equal pass.

The gather table is split into two Shared tensors by source row: the A
half (each core's local rows [0, 4096) = chunks 0..31, 8*4096 = 32768
rows — the exact int16 index limit) and the B half (rows [4096, 6250),
8*2154 rows).  Each half is filled by its own single-writer AllGather:
AG_A fires once GEMM chunk 31 is done (~2/3 through the previous phase,
fully hidden), AG_B at phase end.  The next layer's A-stream gathers are
emitted several chunks ahead of its B-stream gathers, so AG_B's flight
is covered by A-gather work instead of a pipeline bubble.

Per layer, per core (software-pipelined emission, per chunk):
  stage A: dma_gather of table[src] rows for the chunk's A/B edge tiles
     (int16 indices, contiguous fp16 rows, G*128 rows per instruction,
     SWDGE queues round-robin) + one-hot eq build (DVE is_equal)
  stage B: PSUM-accumulated fp16 matmuls ps += eqT.T @ msg
  stage C: relu(dinv_dst * ps [+ bias]) on ACT, residual add, next-layer
     GEMM for the chunk (PE transposes + fp16 matmuls + scaled ACT copy),
     and the half-table AllGathers at chunks 31 / 48
"""

import math
import os

import numpy as np

import concourse.bass as bass
import concourse.mybir as mybir
import concourse.tile as tile
from concourse import bacc
from concourse.bass_utils import run_bass_kernel_spmd
from concourse.masks import make_identity

F32 = mybir.dt.float32
F16 = mybir.dt.float16
I16 = mybir.dt.int16
I32 = mybir.dt.int32

N_NODES = 50000
HID = 256
NCORES = 8
NPC = N_NODES // NCORES          # 6250 nodes per core
NCHUNK = math.ceil(NPC / 128)    # 49 dst chunks per core
G = 8                            # edge tiles per gather instruction (dma_gather tops out at 1024 idxs)
PAD_DST = 255.0                  # dst_local sentinel that matches no iota lane
NLAYERS = 3
NSWDGE_QUEUES = 4                # parallel SWDGE descriptor-gen queues
MM_DT = mybir.dt.float16         # table/message/matmul/h dtype (PSUM accum stays fp32)
DC = 2                           # chunks the epilogue trails the matmul stage
NBUF_A = 10                      # msg/eq ring depth, even stream
NBUF_B = 10                      # msg/eq ring depth, odd stream

_cache = {}


def _pack_stream(flat_idx, flat_dst, NG):
    """flat_* are [NG*G*128] slot arrays in (tile, slot) order.

    Returns packed meta [NG*128, G*5] int32 rows: [G*8 int16 idx | G f32 dst].
    """
    dstT = (
        flat_dst.reshape(NG, G, 128).transpose(0, 2, 1).reshape(NG * 128, G)
    )
    idxT = np.zeros((NG * 128, G * 8), dtype=np.int16)
    vals = flat_idx.reshape(NG, G * 128)
    for g in range(NG):
        a16 = vals[g].reshape(G * 8, 16).T  # [16, G*8]; slot i at [i%16, i//16]
        idxT[g * 128 : (g + 1) * 128] = np.tile(a16, (8, 1))
    meta = np.zeros((NG * 128, G * 4 + G), dtype=np.int32)
    meta[:, : G * 4] = idxT.view(np.int32)
    meta[:, G * 4 : G * 5] = dstT.astype(np.float32).view(np.int32)
    return (meta,)


def _preprocess(edge_index):
    """Edge partitioning by destination + per-core A|B-half stream layouts."""
    src = np.asarray(edge_index[0], dtype=np.int64)
    dst = np.asarray(edge_index[1], dtype=np.int64)
    loops = np.arange(N_NODES, dtype=np.int64)
    s = np.concatenate([src, loops])
    d = np.concatenate([dst, loops])
    deg = np.bincount(d, minlength=N_NODES).astype(np.float32)
    dinv = (1.0 / np.sqrt(np.maximum(deg, 1e-12))).astype(np.float32)
    dinv[deg == 0] = 0.0

    in_a = (s % 2) == 0
    idx_all = s // 2

    edges = []  # [core][chunk] -> ((idxA, dstA), (idxB, dstB))
    cntA = np.zeros((NCORES, NCHUNK), dtype=np.int64)
    cntB = np.zeros((NCORES, NCHUNK), dtype=np.int64)
    for c in range(NCORES):
        lo = c * NPC
        m = (d >= lo) & (d < lo + NPC)
        cs, cd, ca = idx_all[m], (d[m] - lo), in_a[m]
        order = np.argsort(cd, kind="stable")
        cs, cd, ca = cs[order], cd[order], ca[order]
        bounds = np.searchsorted(cd, np.arange(0, NCHUNK + 1) * 128)
        rows = []
        for ch in range(NCHUNK):
            a, b = bounds[ch], bounds[ch + 1]
            es, ed, ea = cs[a:b], cd[a:b] - ch * 128, ca[a:b]
            av = (es[ea], ed[ea])
            bv = (es[~ea], ed[~ea])
            rows.append((av, bv))
            cntA[c, ch] = ea.sum()
            cntB[c, ch] = (~ea).sum()
        edges.append(rows)

    TA = [int(np.ceil(max(cntA[:, ch].max(), 1) / 128)) for ch in range(NCHUNK)]
    TB = [int(np.ceil(max(cntB[:, ch].max(), 1) / 128)) for ch in range(NCHUNK)]
    tilesA, tilesB = int(np.sum(TA)), int(np.sum(TB))
    NGA, NGB = math.ceil(tilesA / G), math.ceil(tilesB / G)
    startA = np.concatenate([[0], np.cumsum(TA)]).astype(int)
    startB = np.concatenate([[0], np.cumsum(TB)]).astype(int)

    per_core = []
    vA = [1] * NGA
    vB = [1] * NGB
    for c in range(NCORES):
        fiA = np.zeros(NGA * G * 128, dtype=np.int64)  # pad idx: row 0
        fdA = np.full(NGA * G * 128, PAD_DST, dtype=np.float32)
        fiB = np.zeros(NGB * G * 128, dtype=np.int64)
        fdB = np.full(NGB * G * 128, PAD_DST, dtype=np.float32)
        for ch in range(NCHUNK):
            (ais, ads), (bis, bds) = edges[c][ch]
            p0 = startA[ch] * 128
            fiA[p0 : p0 + len(ais)] = ais
            fdA[p0 : p0 + len(ads)] = ads
            p0 = startB[ch] * 128
            fiB[p0 : p0 + len(bis)] = bis
            fdB[p0 : p0 + len(bds)] = bds
        per_core.append(_pack_stream(fiA, fdA, NGA) + _pack_stream(fiB, fdB, NGB))
        for vl, fd, ng in ((vA, fdA, NGA), (vB, fdB, NGB)):
            for g in range(ng):
                nz = np.nonzero(fd[g * G * 128 : (g + 1) * G * 128] != PAD_DST)[0]
                v = int(nz[-1]) + 1 if len(nz) else 1
                vl[g] = max(vl[g], v)

    # per-core dinv, laid out [128, NCHUNK] column-per-chunk
    dinv_cols = np.zeros((NCORES, 128, NCHUNK), dtype=np.float32)
    for c in range(NCORES):
        dv = dinv[c * NPC : (c + 1) * NPC]
        pad = np.zeros(NCHUNK * 128, dtype=np.float32)
        pad[: len(dv)] = dv
        dinv_cols[c] = pad.reshape(NCHUNK, 128).T

    sched = (tuple(TA), tuple(TB), tilesA, tilesB, NGA, NGB, tuple(vA), tuple(vB))
    return sched, per_core, dinv_cols


def _build(sched, nlayers=3, has_bias=False):
    TA, TB, tilesA, tilesB, NGA, NGB, VA, VB = sched
    nc = bacc.Bacc(
        "TRN2",
        target_bir_lowering=False,
        debug=False,
        num_devices=NCORES,
        num_swdge_queues=NSWDGE_QUEUES,
    )
    x_ap = nc.dram_tensor("x", [NPC, HID], MM_DT, kind="ExternalInput").ap()
    wts = nc.dram_tensor(
        "wts", [2 * nlayers, 128, HID], MM_DT, kind="ExternalInput"
    ).ap()
    bias = nc.dram_tensor("bias", [nlayers, HID], F32, kind="ExternalInput").ap()
    consts = nc.dram_tensor("consts", [128, 128], F32, kind="ExternalInput").ap()
    dinv_ap = nc.dram_tensor(
        "dinv", [128, NCHUNK], F32, kind="ExternalInput"
    ).ap()
    G5 = G * 5
    metA = nc.dram_tensor(
        "metA", [NGA * 128, G5], I32, kind="ExternalInput"
    ).ap()
    metB = nc.dram_tensor(
        "metB", [NGB * 128, G5], I32, kind="ExternalInput"
    ).ap()
    out_ap = nc.dram_tensor("out", [NPC, HID], F32, kind="ExternalOutput").ap()

    with tile.TileContext(nc) as tc:
        with tc.tile_pool(name="const", bufs=1) as cpool, \
             tc.tile_pool(name="work", bufs=4) as work, \
             tc.tile_pool(name="msgA", bufs=NBUF_A) as msgA, \
             tc.tile_pool(name="msgB", bufs=NBUF_B) as msgB, \
             tc.tile_pool(name="eqA", bufs=NBUF_A) as eqA, \
             tc.tile_pool(name="eqB", bufs=NBUF_B) as eqB, \
             tc.tile_pool(name="ptp", bufs=2, space="PSUM") as ptp, \
             tc.tile_pool(name="ypp", bufs=1, space="PSUM") as ypp, \
             tc.tile_pool(name="psp", bufs=5, space="PSUM") as psp, \
             tc.tile_pool(name="dram", bufs=1, space="DRAM") as dram:

            identity = cpool.tile([128, 128], MM_DT)
            make_identity(nc, identity[:])
            iota_sb = cpool.tile([128, 128], F32)
            nc.sync.dma_start(out=iota_sb[:], in_=consts[:])
            dinv_sb = cpool.tile([128, NCHUNK], F32)
            nc.sync.dma_start(out=dinv_sb[:], in_=dinv_ap[:])

            # whole meta resident in SBUF (single big DMA per stream):
            # gathers/eq builds then have zero runtime DMA dependencies
            met_all = {
                "A": cpool.tile([128, NGA * G5], I32, name="metA_sb"),
                "B": cpool.tile([128, NGB * G5], I32, name="metB_sb"),
            }
            for sname, met_d, ng in (("A", metA, NGA), ("B", metB, NGB)):
                t = met_all[sname]
                nc.sync.dma_start(
                    out=t[:].rearrange("p (g c) -> p g c", g=ng),
                    in_=met_d[:].rearrange("(g p) c -> p g c", g=ng),
                )

            wt_sb = cpool.tile([128, 2 * nlayers * HID], MM_DT)
            for i in range(2 * nlayers):
                nc.sync.dma_start(
                    out=wt_sb[:, i * HID : (i + 1) * HID], in_=wts[i]
                )

            if has_bias:
                bias_row = cpool.tile([1, nlayers * HID], F32)
                ones_sb = cpool.tile([1, 128], F32)
                nc.vector.memset(ones_sb[:], 1.0)
                bias128 = cpool.tile([128, nlayers * HID], F32)
                for l in range(nlayers):
                    nc.sync.dma_start(
                        out=bias_row[:, l * HID : (l + 1) * HID],
                        in_=bias[l : l + 1, :],
                    )
                    bp = ptp.tile([128, HID], F32, tag="pt", name="pt")
                    nc.tensor.matmul(
                        out=bp[:, :],
                        lhsT=ones_sb[:],
                        rhs=bias_row[:, l * HID : (l + 1) * HID],
                        start=True,
                        stop=True,
                    )
                    nc.scalar.copy(
                        out=bias128[:, l * HID : (l + 1) * HID], in_=bp[:, :]
                    )

            # zero-init the msg rings so pad slots never feed NaN bit
            # patterns into the 0-weighted matmul columns
            for pool, n in ((msgA, NBUF_A), (msgB, NBUF_B)):
                for _ in range(n):
                    mz = pool.tile([128, G * HID], MM_DT, tag="msg", name="msg")
                    nc.vector.memset(mz[:], 0)

            # h lives in SBUF as one big tile (single DMA load), fp16
            h_all = cpool.tile([128, NCHUNK * HID], MM_DT, name="h_all")
            nc.sync.dma_start(
                out=h_all[:].rearrange("p (c d) -> p c d", c=NCHUNK)[
                    :, : NPC // 128, :
                ],
                in_=x_ap[: (NPC // 128) * 128, :].rearrange(
                    "(c p) d -> p c d", c=NPC // 128
                ),
            )
            tail = NPC - (NPC // 128) * 128
            if tail:
                nc.sync.dma_start(
                    out=h_all[:tail, (NCHUNK - 1) * HID : NCHUNK * HID],
                    in_=x_ap[(NPC // 128) * 128 :, :],
                )
            h_sb = [h_all[:, c * HID : (c + 1) * HID] for c in range(NCHUNK)]

            y_cs = [
                dram.tile([NPC, HID], MM_DT, name=f"y_c{i}")
                for i in range(nlayers)
            ]
            y_tabs = [
                dram.tile(
                    [NPC * NCORES, HID],
                    MM_DT,
                    addr_space="Shared",
                    name=f"y_table{i}",
                )
                for i in range(nlayers)
            ]

            def gemm_chunk(l, c, pool_pt=None, pool_yp=None):
                """layer-l GEMM chunk c: y rows = dinv * (h_sb[c] @ W_l.T)"""
                pool_pt = pool_pt or ptp
                pool_yp = pool_yp or ypp
                rows = min(128, NPC - c * 128)
                hT = work.tile([128, HID], MM_DT, tag="hT", name="hT")
                for k in range(2):
                    pt = pool_pt.tile([128, 128], MM_DT, tag="pt", name="pt")
                    nc.tensor.transpose(
                        out=pt[:, :rows],
                        in_=h_sb[c][:rows, k * 128 : (k + 1) * 128],
                        identity=identity[:rows, :rows],
                    )
                    nc.scalar.copy(
                        out=hT[:, k * 128 : k * 128 + rows], in_=pt[:, :rows]
                    )
                yp = pool_yp.tile([128, HID], F32, tag="yp", name="yp")
                for k in range(2):
                    nc.tensor.matmul(
                        out=yp[:rows, :],
                        lhsT=hT[:, k * 128 : k * 128 + rows],
                        rhs=wt_sb[:, (2 * l + k) * HID : (2 * l + k + 1) * HID],
                        start=(k == 0),
                        stop=(k == 1),
                    )
                y_sb = work.tile([128, HID], MM_DT, tag="y_sb", name="y_sb")
                nc.scalar.activation(
                    out=y_sb[:rows],
                    in_=yp[:rows, :],
                    func=mybir.ActivationFunctionType.Copy,
                    scale=dinv_sb[:rows, c : c + 1],
                )
                nc.sync.dma_start(
                    out=y_cs[l][c * 128 : c * 128 + rows, :], in_=y_sb[:rows]
                )

            def allgather(l):
                nc.gpsimd.collective_compute(
                    "AllGather",
                    mybir.AluOpType.bypass,
                    replica_groups=[list(range(NCORES))],
                    ins=[y_cs[l][:].opt()],
                    outs=[y_tabs[l][:].opt()],
                )

            for ci in range(NCHUNK):
                gemm_chunk(0, ci)
            allgather(0)

            startA_l = [0]
            for t in TA:
                startA_l.append(startA_l[-1] + t)
            startB_l = [0]
            for t in TB:
                startB_l.append(startB_l[-1] + t)

            for l in range(nlayers):
                stream_info = {
                    "A": (tilesA, NGA, startA_l, y_tabs[l][0::2, :], msgA, eqA, VA),
                    "B": (tilesB, NGB, startB_l, y_tabs[l][1::2, :], msgB, eqB, VB),
                }
                nextg = {"A": 0, "B": 0}
                gbufs = {"A": {}, "B": {}}
                ps_of = {}
                qctr = [0]

                def stage_a(ci, sname):
                    tiles_s, ng_s, starts, view, mpool, epool, vs = stream_info[sname]
                    while (
                        nextg[sname] < ng_s
                        and nextg[sname] * G < starts[ci + 1]
                    ):
                        g = nextg[sname]
                        rem = min(G, tiles_s - g * G)
                        met_sb = met_all[sname][:, g * G5 : (g + 1) * G5]
                        idx_sb = met_sb[:, : G * 4].bitcast(I16)
                        dst_sb = met_sb[:, G * 4 : G * 5].bitcast(F32)
                        msg = mpool.tile(
                            [128, G * HID], MM_DT, tag="msg", name="msg"
                        )
                        v = vs[g]
                        rem_v = (v + 127) // 128
                        nc.gpsimd.dma_gather(
                            out_ap=msg[:, : rem_v * HID].rearrange(
                                "p (g d) -> p g d", g=rem_v
                            ),
                            in_ap=view,
                            idxs_ap=idx_sb[:, : rem_v * 8],
                            num_idxs=v,
                            num_idxs_reg=v,
                            elem_size=HID,
                            elem_step=2 * HID,
                            queue_num=qctr[0] % NSWDGE_QUEUES,
                        )
                        qctr[0] += 1
                        eq = epool.tile(
                            [128, G * 128], MM_DT, tag="eq", name="eq"
                        )
                        eq3 = eq[:, : rem * 128].rearrange(
                            "p (g d) -> p g d", g=rem
                        )
                        nc.vector.tensor_tensor(
                            out=eq3,
                            in0=dst_sb[:, :rem, None].to_broadcast(
                                (128, rem, 128)
                            ),
                            in1=iota_sb[:, None, :].to_broadcast(
                                (128, rem, 128)
                            ),
                            op=mybir.AluOpType.is_equal,
                        )
                        gbufs[sname][g] = (msg, eq)
                        nextg[sname] += 1

                def stage_b(ci):
                    ntot = TA[ci] + TB[ci]
                    ps = psp.tile([128, HID], F32, tag="ps", name="ps")
                    jj = 0
                    for sname, T_s, starts in (
                        ("A", TA, startA_l),
                        ("B", TB, startB_l),
                    ):
                        for t in range(T_s[ci]):
                            g, col = divmod(starts[ci] + t, G)
                            msg, eq = gbufs[sname][g]
                            nc.tensor.matmul(
                                out=ps[:, :],
                                lhsT=eq[:, col * 128 : (col + 1) * 128],
                                rhs=msg[:, col * HID : (col + 1) * HID],
                                start=(jj == 0),
                                stop=(jj == ntot - 1),
                            )
                            jj += 1
                    ps_of[ci] = ps

                def stage_c(ci):
                    crows = min(128, NPC - ci * 128)
                    ps = ps_of.pop(ci)
                    if has_bias:
                        t_sb = work.tile([128, HID], F32, tag="o_sb", name="t_sb")
                        nc.vector.scalar_tensor_tensor(
                            out=t_sb[:crows],
                            in0=ps[:crows, :],
                            scalar=dinv_sb[:crows, ci : ci + 1],
                            in1=bias128[:crows, l * HID : (l + 1) * HID],
                            op0=mybir.AluOpType.mult,
                            op1=mybir.AluOpType.add,
                        )
                        relu_in, relu_scale = t_sb, 1.0
                    else:
                        relu_in, relu_scale = ps, dinv_sb[:crows, ci : ci + 1]
                    if l == 0:
                        nc.scalar.activation(
                            out=h_sb[ci][:crows],
                            in_=relu_in[:crows, :],
                            func=mybir.ActivationFunctionType.Relu,
                            scale=relu_scale,
                        )
                    else:
                        odt = F32 if l == nlayers - 1 else MM_DT
                        o_sb = work.tile([128, HID], odt, tag="o_sb", name="o_sb")
                        nc.scalar.activation(
                            out=o_sb[:crows],
                            in_=relu_in[:crows, :],
                            func=mybir.ActivationFunctionType.Relu,
                            scale=relu_scale,
                        )
                        if l < nlayers - 1:
                            nc.vector.tensor_add(
                                out=h_sb[ci][:crows],
                                in0=o_sb[:crows],
                                in1=h_sb[ci][:crows],
                            )
                        else:
                            nc.vector.tensor_add(
                                out=o_sb[:crows],
                                in0=o_sb[:crows],
                                in1=h_sb[ci][:crows],
                            )
                            nc.sync.dma_start(
                                out=out_ap[ci * 128 : ci * 128 + crows, :],
                                in_=o_sb[:crows],
                            )
                    if l + 1 < nlayers:
                        gemm_chunk(l + 1, ci)
                        if ci == NCHUNK - 1:
                            allgather(l + 1)

                NIT = NCHUNK + 1 + DC
                for it in range(NIT):
                    if it < NCHUNK:
                        stage_a(it, "A")
                        stage_a(it, "B")
                    if 1 <= it < NCHUNK + 1:
                        stage_b(it - 1)
                    if it >= 1 + DC:
                        stage_c(it - 1 - DC)

    nc.compile()
    return nc


def _consts_array():
    consts = np.zeros((128, 128), dtype=np.float32)
    consts[:, :] = np.arange(128, dtype=np.float32)[None, :]
    return consts


def kernel(x, edge_index, W0, b0, W1, b1, W2, b2):
    x = np.asarray(x, dtype=np.float32)
    edge_index = np.asarray(edge_index)
    Ws = [np.asarray(w, dtype=np.float32) for w in (W0, W1, W2)]
    bs = [np.asarray(b, dtype=np.float32) for b in (b0, b1, b2)]
    has_bias = any(np.any(b != 0) for b in bs)

    sched, per_core, dinv_cols = _preprocess(edge_index)

    key = (sched, NLAYERS, has_bias)
    if key not in _cache:
        _cache[key] = _build(sched, nlayers=NLAYERS, has_bias=has_bias)
    nc = _cache[key]

    wts = np.stack(
        [w.T[k * 128 : (k + 1) * 128, :] for w in Ws for k in range(2)]
    ).astype(np.float16)
    bias_arr = np.stack(bs).astype(np.float32)
    consts = _consts_array()

    in_maps = []
    for c in range(NCORES):
        mA, mB = per_core[c]
        in_maps.append(
            {
                "x": np.ascontiguousarray(x[c * NPC : (c + 1) * NPC]).astype(np.float16),
                "wts": wts,
                "bias": bias_arr,
                "consts": consts,
                "dinv": np.ascontiguousarray(dinv_cols[c]),
                "metA": mA,
                "metB": mB,
            }
        )

    trace = bool(int(os.environ.get("GCN_TRACE", "0")))
    res = run_bass_kernel_spmd(
        nc, in_maps, core_ids=list(range(NCORES)), trace=trace
    )
    if trace:
        kernel.last_exec_time_ns = res.exec_time_ns
        kernel.last_results = res
    out = np.concatenate([res.results[c]["out"] for c in range(NCORES)], axis=0)
    return out


# revision 39
# speedup vs baseline: 1.4759x; 1.1380x over previous
"""3-layer GCN (PyG-style GCNConv with self-loops + symmetric norm) on 8
Trainium2 NeuronCores.

Distribution (1D graph partitioning):
  - nodes split into 8 contiguous blocks of 6250 rows, one per core
  - edges partitioned by destination core, sorted by destination node
  - 256x256 weights replicated on every core

The symmetric norm dinv[src]*dinv[dst] is factored out of the per-edge
selection matrix: the src factor is folded into the gathered table rows
(table[s] = dinv[s] * y[s], applied for free by the ACT-engine copy that
drains the GEMM PSUM), and the dst factor is applied by the ACT-engine
ReLU epilogue (activation scale operand).  The selection matrix is then a
pure one-hot built in a single DVE is_equal pass.

The gather table is split into two Shared tensors by source row: the A
half (each core's local rows [0, 4096) = chunks 0..31, 8*4096 = 32768
rows — the exact int16 index limit) and the B half (rows [4096, 6250),
8*2154 rows).  Each half is filled by its own single-writer AllGather:
AG_A fires once GEMM chunk 31 is done (~2/3 through the previous phase,
fully hidden), AG_B at phase end.  The next layer's A-stream gathers are
emitted several chunks ahead of its B-stream gathers, so AG_B's flight
is covered by A-gather work instead of a pipeline bubble.

Per layer, per core (software-pipelined emission, per chunk):
  stage A: dma_gather of table[src] rows for the chunk's A/B edge tiles
     (int16 indices, contiguous fp16 rows, G*128 rows per instruction,
     SWDGE queues round-robin) + one-hot eq build (DVE is_equal)
  stage B: PSUM-accumulated fp16 matmuls ps += eqT.T @ msg
  stage C: relu(dinv_dst * ps [+ bias]) on ACT, residual add, next-layer
     GEMM for the chunk (PE transposes + fp16 matmuls + scaled ACT copy),
     and the half-table AllGathers at chunks 31 / 48
"""

import math
import os

import numpy as np

import concourse.bass as bass
import concourse.mybir as mybir
import concourse.tile as tile
from concourse import bacc
from concourse.bass_utils import run_bass_kernel_spmd
from concourse.masks import make_identity

F32 = mybir.dt.float32
F16 = mybir.dt.float16
I16 = mybir.dt.int16
I32 = mybir.dt.int32

N_NODES = 50000
HID = 256
NCORES = 8
NPC = N_NODES // NCORES          # 6250 nodes per core
NCHUNK = math.ceil(NPC / 128)    # 49 dst chunks per core
G = 8                            # edge tiles per gather instruction (dma_gather tops out at 1024 idxs)
PAD_DST = 255.0                  # dst_local sentinel that matches no iota lane
NLAYERS = 3
NSWDGE_QUEUES = 4                # parallel SWDGE descriptor-gen queues
MM_DT = mybir.dt.float16         # h/GEMM dtype (PSUM accum stays fp32)
# Per-layer table/message/eq dtype. Layer 0's table noise passes through
# two further layers of aggregation, so it stays fp16; layers 1-2 use
# fp8e4m3 (halves their gather + AllGather bytes; PSUM accumulation is
# fp32, final rel err ~1e-2 vs the 2e-2 gate).
_TB_MAP = {"16": mybir.dt.float16, "8": mybir.dt.float8e4}
TBS = [
    _TB_MAP[t]
    for t in os.environ.get("GCN_TB", "16,8,8").split(",")
]
DC = 2                           # chunks the epilogue trails the matmul stage
NBUF_A = 10                      # msg/eq ring depth, even stream
NBUF_B = 10                      # msg/eq ring depth, odd stream

_cache = {}


def _pack_stream(flat_idx, flat_dst, NG):
    """flat_* are [NG*G*128] slot arrays in (tile, slot) order.

    Returns packed meta [NG*128, G*5] int32 rows: [G*8 int16 idx | G f32 dst].
    """
    dstT = (
        flat_dst.reshape(NG, G, 128).transpose(0, 2, 1).reshape(NG * 128, G)
    )
    idxT = np.zeros((NG * 128, G * 8), dtype=np.int16)
    vals = flat_idx.reshape(NG, G * 128)
    for g in range(NG):
        a16 = vals[g].reshape(G * 8, 16).T  # [16, G*8]; slot i at [i%16, i//16]
        idxT[g * 128 : (g + 1) * 128] = np.tile(a16, (8, 1))
    meta = np.zeros((NG * 128, G * 4 + G), dtype=np.int32)
    meta[:, : G * 4] = idxT.view(np.int32)
    meta[:, G * 4 : G * 5] = dstT.astype(np.float32).view(np.int32)
    return (meta,)


def _preprocess(edge_index):
    """Edge partitioning by destination + per-core A|B-half stream layouts."""
    src = np.asarray(edge_index[0], dtype=np.int64)
    dst = np.asarray(edge_index[1], dtype=np.int64)
    loops = np.arange(N_NODES, dtype=np.int64)
    s = np.concatenate([src, loops])
    d = np.concatenate([dst, loops])
    deg = np.bincount(d, minlength=N_NODES).astype(np.float32)
    dinv = (1.0 / np.sqrt(np.maximum(deg, 1e-12))).astype(np.float32)
    dinv[deg == 0] = 0.0

    in_a = (s % 2) == 0
    idx_all = s // 2

    edges = []  # [core][chunk] -> ((idxA, dstA), (idxB, dstB))
    cntA = np.zeros((NCORES, NCHUNK), dtype=np.int64)
    cntB = np.zeros((NCORES, NCHUNK), dtype=np.int64)
    for c in range(NCORES):
        lo = c * NPC
        m = (d >= lo) & (d < lo + NPC)
        cs, cd, ca = idx_all[m], (d[m] - lo), in_a[m]
        order = np.argsort(cd, kind="stable")
        cs, cd, ca = cs[order], cd[order], ca[order]
        bounds = np.searchsorted(cd, np.arange(0, NCHUNK + 1) * 128)
        rows = []
        for ch in range(NCHUNK):
            a, b = bounds[ch], bounds[ch + 1]
            es, ed, ea = cs[a:b], cd[a:b] - ch * 128, ca[a:b]
            av = (es[ea], ed[ea])
            bv = (es[~ea], ed[~ea])
            rows.append((av, bv))
            cntA[c, ch] = ea.sum()
            cntB[c, ch] = (~ea).sum()
        edges.append(rows)

    TA = [int(np.ceil(max(cntA[:, ch].max(), 1) / 128)) for ch in range(NCHUNK)]
    TB = [int(np.ceil(max(cntB[:, ch].max(), 1) / 128)) for ch in range(NCHUNK)]
    tilesA, tilesB = int(np.sum(TA)), int(np.sum(TB))
    NGA, NGB = math.ceil(tilesA / G), math.ceil(tilesB / G)
    startA = np.concatenate([[0], np.cumsum(TA)]).astype(int)
    startB = np.concatenate([[0], np.cumsum(TB)]).astype(int)

    per_core = []
    vA = [1] * NGA
    vB = [1] * NGB
    for c in range(NCORES):
        fiA = np.zeros(NGA * G * 128, dtype=np.int64)  # pad idx: row 0
        fdA = np.full(NGA * G * 128, PAD_DST, dtype=np.float32)
        fiB = np.zeros(NGB * G * 128, dtype=np.int64)
        fdB = np.full(NGB * G * 128, PAD_DST, dtype=np.float32)
        for ch in range(NCHUNK):
            (ais, ads), (bis, bds) = edges[c][ch]
            p0 = startA[ch] * 128
            fiA[p0 : p0 + len(ais)] = ais
            fdA[p0 : p0 + len(ads)] = ads
            p0 = startB[ch] * 128
            fiB[p0 : p0 + len(bis)] = bis
            fdB[p0 : p0 + len(bds)] = bds
        per_core.append(_pack_stream(fiA, fdA, NGA) + _pack_stream(fiB, fdB, NGB))
        for vl, fd, ng in ((vA, fdA, NGA), (vB, fdB, NGB)):
            for g in range(ng):
                nz = np.nonzero(fd[g * G * 128 : (g + 1) * G * 128] != PAD_DST)[0]
                v = int(nz[-1]) + 1 if len(nz) else 1
                vl[g] = max(vl[g], v)

    # per-core dinv, laid out [128, NCHUNK] column-per-chunk
    dinv_cols = np.zeros((NCORES, 128, NCHUNK), dtype=np.float32)
    for c in range(NCORES):
        dv = dinv[c * NPC : (c + 1) * NPC]
        pad = np.zeros(NCHUNK * 128, dtype=np.float32)
        pad[: len(dv)] = dv
        dinv_cols[c] = pad.reshape(NCHUNK, 128).T

    sched = (tuple(TA), tuple(TB), tilesA, tilesB, NGA, NGB, tuple(vA), tuple(vB))
    return sched, per_core, dinv_cols


def _build(sched, nlayers=3, has_bias=False):
    TA, TB, tilesA, tilesB, NGA, NGB, VA, VB = sched
    nc = bacc.Bacc(
        "TRN2",
        target_bir_lowering=False,
        debug=False,
        num_devices=NCORES,
        num_swdge_queues=NSWDGE_QUEUES,
    )
    x_ap = nc.dram_tensor("x", [NPC, HID], MM_DT, kind="ExternalInput").ap()
    wts = nc.dram_tensor(
        "wts", [2 * nlayers, 128, HID], MM_DT, kind="ExternalInput"
    ).ap()
    bias = nc.dram_tensor("bias", [nlayers, HID], F32, kind="ExternalInput").ap()
    consts = nc.dram_tensor("consts", [128, 128], F32, kind="ExternalInput").ap()
    dinv_ap = nc.dram_tensor(
        "dinv", [128, NCHUNK], F32, kind="ExternalInput"
    ).ap()
    G5 = G * 5
    metA = nc.dram_tensor(
        "metA", [NGA * 128, G5], I32, kind="ExternalInput"
    ).ap()
    metB = nc.dram_tensor(
        "metB", [NGB * 128, G5], I32, kind="ExternalInput"
    ).ap()
    out_ap = nc.dram_tensor("out", [NPC, HID], F32, kind="ExternalOutput").ap()

    with tile.TileContext(nc) as tc:
        with tc.tile_pool(name="const", bufs=1) as cpool, \
             tc.tile_pool(name="work", bufs=4) as work, \
             tc.tile_pool(name="msgA", bufs=NBUF_A) as msgA, \
             tc.tile_pool(name="msgB", bufs=NBUF_B) as msgB, \
             tc.tile_pool(name="eqA", bufs=NBUF_A) as eqA, \
             tc.tile_pool(name="eqB", bufs=NBUF_B) as eqB, \
             tc.tile_pool(name="ptp", bufs=2, space="PSUM") as ptp, \
             tc.tile_pool(name="ypp", bufs=1, space="PSUM") as ypp, \
             tc.tile_pool(name="psp", bufs=5, space="PSUM") as psp, \
             tc.tile_pool(name="dram", bufs=1, space="DRAM") as dram:

            identity = cpool.tile([128, 128], MM_DT)
            make_identity(nc, identity[:])
            iota_sb = cpool.tile([128, 128], F32)
            nc.sync.dma_start(out=iota_sb[:], in_=consts[:])
            dinv_sb = cpool.tile([128, NCHUNK], F32)
            nc.sync.dma_start(out=dinv_sb[:], in_=dinv_ap[:])

            # whole meta resident in SBUF (single big DMA per stream):
            # gathers/eq builds then have zero runtime DMA dependencies
            met_all = {
                "A": cpool.tile([128, NGA * G5], I32, name="metA_sb"),
                "B": cpool.tile([128, NGB * G5], I32, name="metB_sb"),
            }
            for sname, met_d, ng in (("A", metA, NGA), ("B", metB, NGB)):
                t = met_all[sname]
                nc.sync.dma_start(
                    out=t[:].rearrange("p (g c) -> p g c", g=ng),
                    in_=met_d[:].rearrange("(g p) c -> p g c", g=ng),
                )

            wt_sb = cpool.tile([128, 2 * nlayers * HID], MM_DT)
            for i in range(2 * nlayers):
                nc.sync.dma_start(
                    out=wt_sb[:, i * HID : (i + 1) * HID], in_=wts[i]
                )

            if has_bias:
                bias_row = cpool.tile([1, nlayers * HID], F32)
                ones_sb = cpool.tile([1, 128], F32)
                nc.vector.memset(ones_sb[:], 1.0)
                bias128 = cpool.tile([128, nlayers * HID], F32)
                for l in range(nlayers):
                    nc.sync.dma_start(
                        out=bias_row[:, l * HID : (l + 1) * HID],
                        in_=bias[l : l + 1, :],
                    )
                    bp = ptp.tile([128, HID], F32, tag="pt", name="pt")
                    nc.tensor.matmul(
                        out=bp[:, :],
                        lhsT=ones_sb[:],
                        rhs=bias_row[:, l * HID : (l + 1) * HID],
                        start=True,
                        stop=True,
                    )
                    nc.scalar.copy(
                        out=bias128[:, l * HID : (l + 1) * HID], in_=bp[:, :]
                    )

            # zero-init the msg rings so pad slots never feed NaN bit
            # patterns into the 0-weighted matmul columns
            for pool, n in ((msgA, NBUF_A), (msgB, NBUF_B)):
                for _ in range(n):
                    mz = pool.tile([128, G * HID], F16, tag="msg", name="msg")
                    nc.vector.memset(mz[:], 0)

            # h lives in SBUF as one big tile (single DMA load), fp16
            h_all = cpool.tile([128, NCHUNK * HID], MM_DT, name="h_all")
            nc.sync.dma_start(
                out=h_all[:].rearrange("p (c d) -> p c d", c=NCHUNK)[
                    :, : NPC // 128, :
                ],
                in_=x_ap[: (NPC // 128) * 128, :].rearrange(
                    "(c p) d -> p c d", c=NPC // 128
                ),
            )
            tail = NPC - (NPC // 128) * 128
            if tail:
                nc.sync.dma_start(
                    out=h_all[:tail, (NCHUNK - 1) * HID : NCHUNK * HID],
                    in_=x_ap[(NPC // 128) * 128 :, :],
                )
            h_sb = [h_all[:, c * HID : (c + 1) * HID] for c in range(NCHUNK)]

            y_cs = [
                dram.tile([NPC, HID], TBS[i], name=f"y_c{i}")
                for i in range(nlayers)
            ]
            y_tabs = [
                dram.tile(
                    [NPC * NCORES, HID],
                    TBS[i],
                    addr_space="Shared",
                    name=f"y_table{i}",
                )
                for i in range(nlayers)
            ]

            def gemm_chunk(l, c, pool_pt=None, pool_yp=None):
                """layer-l GEMM chunk c: y rows = dinv * (h_sb[c] @ W_l.T)"""
                pool_pt = pool_pt or ptp
                pool_yp = pool_yp or ypp
                rows = min(128, NPC - c * 128)
                hT = work.tile([128, HID], MM_DT, tag="hT", name="hT")
                for k in range(2):
                    pt = pool_pt.tile([128, 128], MM_DT, tag="pt", name="pt")
                    nc.tensor.transpose(
                        out=pt[:, :rows],
                        in_=h_sb[c][:rows, k * 128 : (k + 1) * 128],
                        identity=identity[:rows, :rows],
                    )
                    nc.scalar.copy(
                        out=hT[:, k * 128 : k * 128 + rows], in_=pt[:, :rows]
                    )
                yp = pool_yp.tile([128, HID], F32, tag="yp", name="yp")
                for k in range(2):
                    nc.tensor.matmul(
                        out=yp[:rows, :],
                        lhsT=hT[:, k * 128 : k * 128 + rows],
                        rhs=wt_sb[:, (2 * l + k) * HID : (2 * l + k + 1) * HID],
                        start=(k == 0),
                        stop=(k == 1),
                    )
                y_sb = work.tile([128, HID], TBS[l], tag="y_sb", name="y_sb")
                nc.scalar.activation(
                    out=y_sb[:rows],
                    in_=yp[:rows, :],
                    func=mybir.ActivationFunctionType.Copy,
                    scale=dinv_sb[:rows, c : c + 1],
                )
                nc.sync.dma_start(
                    out=y_cs[l][c * 128 : c * 128 + rows, :], in_=y_sb[:rows]
                )

            def allgather(l):
                nc.gpsimd.collective_compute(
                    "AllGather",
                    mybir.AluOpType.bypass,
                    replica_groups=[list(range(NCORES))],
                    ins=[y_cs[l][:].opt()],
                    outs=[y_tabs[l][:].opt()],
                )

            for ci in range(NCHUNK):
                gemm_chunk(0, ci)
            allgather(0)

            startA_l = [0]
            for t in TA:
                startA_l.append(startA_l[-1] + t)
            startB_l = [0]
            for t in TB:
                startB_l.append(startB_l[-1] + t)

            for l in range(nlayers):
                stream_info = {
                    "A": (tilesA, NGA, startA_l, y_tabs[l][0::2, :], msgA, eqA, VA),
                    "B": (tilesB, NGB, startB_l, y_tabs[l][1::2, :], msgB, eqB, VB),
                }
                nextg = {"A": 0, "B": 0}
                gbufs = {"A": {}, "B": {}}
                ps_of = {}
                qctr = [0]

                def stage_a(ci, sname):
                    tiles_s, ng_s, starts, view, mpool, epool, vs = stream_info[sname]
                    while (
                        nextg[sname] < ng_s
                        and nextg[sname] * G < starts[ci + 1]
                    ):
                        g = nextg[sname]
                        rem = min(G, tiles_s - g * G)
                        met_sb = met_all[sname][:, g * G5 : (g + 1) * G5]
                        idx_sb = met_sb[:, : G * 4].bitcast(I16)
                        dst_sb = met_sb[:, G * 4 : G * 5].bitcast(F32)
                        msg = mpool.tile(
                            [128, G * HID], TBS[l], tag="msg", name="msg"
                        )
                        # no trailing trim: every slot must be rewritten
                        # each layer so stale bytes never alias fp8 NaNs
                        v = rem * 128
                        rem_v = rem
                        nc.gpsimd.dma_gather(
                            out_ap=msg[:, : rem_v * HID].rearrange(
                                "p (g d) -> p g d", g=rem_v
                            ),
                            in_ap=view,
                            idxs_ap=idx_sb[:, : rem_v * 8],
                            num_idxs=v,
                            num_idxs_reg=v,
                            elem_size=HID,
                            elem_step=2 * HID,
                            queue_num=qctr[0] % NSWDGE_QUEUES,
                        )
                        qctr[0] += 1
                        eq = epool.tile(
                            [128, G * 128], TBS[l], tag="eq", name="eq"
                        )
                        eq3 = eq[:, : rem * 128].rearrange(
                            "p (g d) -> p g d", g=rem
                        )
                        nc.vector.tensor_tensor(
                            out=eq3,
                            in0=dst_sb[:, :rem, None].to_broadcast(
                                (128, rem, 128)
                            ),
                            in1=iota_sb[:, None, :].to_broadcast(
                                (128, rem, 128)
                            ),
                            op=mybir.AluOpType.is_equal,
                        )
                        gbufs[sname][g] = (msg, eq)
                        nextg[sname] += 1

                def stage_b(ci):
                    ntot = TA[ci] + TB[ci]
                    ps = psp.tile([128, HID], F32, tag="ps", name="ps")
                    jj = 0
                    for sname, T_s, starts in (
                        ("A", TA, startA_l),
                        ("B", TB, startB_l),
                    ):
                        for t in range(T_s[ci]):
                            g, col = divmod(starts[ci] + t, G)
                            msg, eq = gbufs[sname][g]
                            nc.tensor.matmul(
                                out=ps[:, :],
                                lhsT=eq[:, col * 128 : (col + 1) * 128],
                                rhs=msg[:, col * HID : (col + 1) * HID],
                                start=(jj == 0),
                                stop=(jj == ntot - 1),
                            )
                            jj += 1
                    ps_of[ci] = ps

                def stage_c(ci):
                    crows = min(128, NPC - ci * 128)
                    ps = ps_of.pop(ci)
                    if has_bias:
                        t_sb = work.tile([128, HID], F32, tag="o_sb", name="t_sb")
                        nc.vector.scalar_tensor_tensor(
                            out=t_sb[:crows],
                            in0=ps[:crows, :],
                            scalar=dinv_sb[:crows, ci : ci + 1],
                            in1=bias128[:crows, l * HID : (l + 1) * HID],
                            op0=mybir.AluOpType.mult,
                            op1=mybir.AluOpType.add,
                        )
                        relu_in, relu_scale = t_sb, 1.0
                    else:
                        relu_in, relu_scale = ps, dinv_sb[:crows, ci : ci + 1]
                    if l == 0:
                        nc.scalar.activation(
                            out=h_sb[ci][:crows],
                            in_=relu_in[:crows, :],
                            func=mybir.ActivationFunctionType.Relu,
                            scale=relu_scale,
                        )
                    else:
                        odt = F32 if l == nlayers - 1 else MM_DT
                        o_sb = work.tile([128, HID], odt, tag="o_sb", name="o_sb")
                        nc.scalar.activation(
                            out=o_sb[:crows],
                            in_=relu_in[:crows, :],
                            func=mybir.ActivationFunctionType.Relu,
                            scale=relu_scale,
                        )
                        if l < nlayers - 1:
                            nc.vector.tensor_add(
                                out=h_sb[ci][:crows],
                                in0=o_sb[:crows],
                                in1=h_sb[ci][:crows],
                            )
                        else:
                            nc.vector.tensor_add(
                                out=o_sb[:crows],
                                in0=o_sb[:crows],
                                in1=h_sb[ci][:crows],
                            )
                            nc.sync.dma_start(
                                out=out_ap[ci * 128 : ci * 128 + crows, :],
                                in_=o_sb[:crows],
                            )
                    if l + 1 < nlayers:
                        gemm_chunk(l + 1, ci)
                        if ci == NCHUNK - 1:
                            allgather(l + 1)

                NIT = NCHUNK + 1 + DC
                for it in range(NIT):
                    if it < NCHUNK:
                        stage_a(it, "A")
                        stage_a(it, "B")
                    if 1 <= it < NCHUNK + 1:
                        stage_b(it - 1)
                    if it >= 1 + DC:
                        stage_c(it - 1 - DC)

    nc.compile()
    return nc


def _consts_array():
    consts = np.zeros((128, 128), dtype=np.float32)
    consts[:, :] = np.arange(128, dtype=np.float32)[None, :]
    return consts


def kernel(x, edge_index, W0, b0, W1, b1, W2, b2):
    x = np.asarray(x, dtype=np.float32)
    edge_index = np.asarray(edge_index)
    Ws = [np.asarray(w, dtype=np.float32) for w in (W0, W1, W2)]
    bs = [np.asarray(b, dtype=np.float32) for b in (b0, b1, b2)]
    has_bias = any(np.any(b != 0) for b in bs)

    sched, per_core, dinv_cols = _preprocess(edge_index)

    key = (sched, NLAYERS, has_bias)
    if key not in _cache:
        _cache[key] = _build(sched, nlayers=NLAYERS, has_bias=has_bias)
    nc = _cache[key]

    wts = np.stack(
        [w.T[k * 128 : (k + 1) * 128, :] for w in Ws for k in range(2)]
    ).astype(np.float16)
    bias_arr = np.stack(bs).astype(np.float32)
    consts = _consts_array()

    in_maps = []
    for c in range(NCORES):
        mA, mB = per_core[c]
        in_maps.append(
            {
                "x": np.ascontiguousarray(x[c * NPC : (c + 1) * NPC]).astype(np.float16),
                "wts": wts,
                "bias": bias_arr,
                "consts": consts,
                "dinv": np.ascontiguousarray(dinv_cols[c]),
                "metA": mA,
                "metB": mB,
            }
        )

    trace = bool(int(os.environ.get("GCN_TRACE", "0")))
    res = run_bass_kernel_spmd(
        nc, in_maps, core_ids=list(range(NCORES)), trace=trace
    )
    if trace:
        kernel.last_exec_time_ns = res.exec_time_ns
        kernel.last_results = res
    out = np.concatenate([res.results[c]["out"] for c in range(NCORES)], axis=0)
    return out
